# revision 6
# baseline (speedup 1.0000x reference)
"""Distributed attention kernel for 8 TRN2 NeuronCores.

Problem: cross-attention (q from target, k/v from reference) with
B=2, N=M=2048, C=1024, H=16 heads, hd=64, followed by an output
projection with bias.

Sharding (data + head parallel):
  core c in 0..7 owns heads {2c, 2c+1} for BOTH batches. It computes
  K^T/Q^T/V for its heads, attention (softmax over keys), producing
  x_local^T [128ch, 2048m] per batch. A single global AllToAll then
  redistributes so core c owns output rows (batch c//4,
  m-block (c%4)*512) with ALL 1024 channels; core c applies the full
  Wproj ([1024,1024], replicated) + bias to its row-block.

Everything on-device runs in a transposed layout ([channels, seq]) so
that no transposes of the big activations are needed: the host
pre-transposes the inputs, and the host re-transposes each core's
[1024, 512] output block. Matmuls run in bf16 (f32 PSUM accumulation);
softmax denominators come free as a ones-column appended to V.
"""

import functools

import numpy as np

B = 2
N = 2048  # reference rows (keys)
M = 2048  # target rows (queries)
C = 1024
H = 16
HD = 64
NCORES = 8
HPC = 2  # heads per core
CHPC = HPC * HD  # 128 channels per core
MBLK = M // 4  # 512 output rows owned per core
NT = 512  # n/m tile for projections / S^T
KC = N // 128  # 16 key chunks
CC = C // 128  # 8 contraction chunks


@functools.lru_cache(maxsize=1)
def _build():
    import concourse.bacc as bacc
    import concourse.mybir as mybir
    import concourse.tile as tile
    from concourse.masks import make_identity

    fp32 = mybir.dt.float32
    bf16 = mybir.dt.bfloat16
    AF = mybir.ActivationFunctionType

    nc = bacc.Bacc("TRN2", target_bir_lowering=False, debug=False, num_devices=NCORES)

    xrefT = nc.dram_tensor("xrefT", [B, C, N], bf16, kind="ExternalInput")
    xtgtT = nc.dram_tensor("xtgtT", [B, C, M], bf16, kind="ExternalInput")
    wq = nc.dram_tensor("wq", [C, CHPC], bf16, kind="ExternalInput")
    wk = nc.dram_tensor("wk", [C, CHPC], bf16, kind="ExternalInput")
    wv = nc.dram_tensor("wv", [C, CHPC], bf16, kind="ExternalInput")
    wproj = nc.dram_tensor("wproj", [C, C], bf16, kind="ExternalInput")
    bproj = nc.dram_tensor("bproj", [C], fp32, kind="ExternalInput")
    out = nc.dram_tensor("out", [C, MBLK], fp32, kind="ExternalOutput")

    with tile.TileContext(nc) as tc:
        with (
            tc.tile_pool(name="consts", bufs=1) as consts,
            tc.tile_pool(name="wpool", bufs=1) as wpool,
            tc.tile_pool(name="xpool", bufs=10) as xpool,
            tc.tile_pool(name="kqv", bufs=1) as kqv,
            tc.tile_pool(name="epool", bufs=3) as epool,
            tc.tile_pool(name="spool", bufs=4) as spool,
            tc.tile_pool(name="stage", bufs=2) as stpool,
            tc.tile_pool(name="ppool", bufs=2) as ppool,
            tc.tile_pool(name="psS", bufs=2, space="PSUM") as psS,
            tc.tile_pool(name="psO", bufs=2, space="PSUM") as psO,
            tc.tile_pool(name="psQ", bufs=2, space="PSUM") as psQ,
            tc.tile_pool(name="dram", bufs=1, space="DRAM") as dpool,
        ):
            ident = consts.tile([128, 128], bf16)
            make_identity(nc, ident[:])

            # ---- weights to SBUF ----
            wq_sb = wpool.tile([128, CC, CHPC], bf16)
            wk_sb = wpool.tile([128, CC, CHPC], bf16)
            wv_sb = wpool.tile([128, CC, CHPC], bf16)
            for cc in range(CC):
                nc.sync.dma_start(wq_sb[:, cc, :], wq[cc * 128:(cc + 1) * 128, :])
                nc.sync.dma_start(wk_sb[:, cc, :], wk[cc * 128:(cc + 1) * 128, :])
                nc.sync.dma_start(wv_sb[:, cc, :], wv[cc * 128:(cc + 1) * 128, :])
            wp_sb = wpool.tile([128, CC, C], bf16)
            for cc in range(CC):
                nc.sync.dma_start(wp_sb[:, cc, :], wproj[cc * 128:(cc + 1) * 128, :])
            bias_sb = wpool.tile([128, CC], fp32)
            nc.sync.dma_start(bias_sb[:], bproj.ap().rearrange("(a p) -> p a", p=128))

            # ---- K^T, Q^T, V per batch ----
            kT = []
            qT = []
            vA = []
            for b in range(B):
                kT.append(kqv.tile([128, N], bf16, tag=f"kT{b}", name=f"kT{b}"))
                qT.append(kqv.tile([128, M], bf16, tag=f"qT{b}", name=f"qT{b}"))
                # V augmented with a ones column per head: [kchunk][head][65]
                vA.append(kqv.tile([128, KC, HPC, HD + 1], bf16, tag=f"vA{b}", name=f"vA{b}"))
                nc.vector.memset(vA[b][:, :, :, HD:HD + 1], 1.0)

            for b in range(B):
                xr = []
                for cc in range(CC):
                    t = xpool.tile([128, N], bf16, tag="x", name=f"xr{b}_{cc}")
                    nc.sync.dma_start(t[:], xrefT[b, cc * 128:(cc + 1) * 128, :])
                    xr.append(t)
                # K^T [128ch, N]
                for nt in range(N // NT):
                    ps = psQ.tile([128, NT], fp32, tag="ps")
                    for cc in range(CC):
                        nc.tensor.matmul(
                            ps[:],
                            lhsT=wk_sb[:, cc, :],
                            rhs=xr[cc][:, nt * NT:(nt + 1) * NT],
                            start=(cc == 0),
                            stop=(cc == CC - 1),
                        )
                    nc.vector.tensor_copy(kT[b][:, nt * NT:(nt + 1) * NT], ps[:])
                # V [n, 128ch] in natural layout, written per key-chunk
                for kc in range(KC):
                    ps = psQ.tile([128, NT], fp32, tag="ps")
                    for cc in range(CC):
                        nc.tensor.matmul(
                            ps[:, 0:CHPC],
                            lhsT=xr[cc][:, kc * 128:(kc + 1) * 128],
                            rhs=wv_sb[:, cc, :],
                            start=(cc == 0),
                            stop=(cc == CC - 1),
                        )
                    nc.vector.tensor_copy(
                        vA[b][:, kc, :, 0:HD],
                        ps[:, 0:CHPC].rearrange("p (h d) -> p h d", h=HPC),
                    )
                # Q^T [128ch, M]
                xt = []
                for cc in range(CC):
                    t = xpool.tile([128, M], bf16, tag="x", name=f"xt{b}_{cc}")
                    nc.sync.dma_start(t[:], xtgtT[b, cc * 128:(cc + 1) * 128, :])
                    xt.append(t)
                for nt in range(M // NT):
                    ps = psQ.tile([128, NT], fp32, tag="ps")
                    for cc in range(CC):
                        nc.tensor.matmul(
                            ps[:],
                            lhsT=wq_sb[:, cc, :],
                            rhs=xt[cc][:, nt * NT:(nt + 1) * NT],
                            start=(cc == 0),
                            stop=(cc == CC - 1),
                        )
                    nc.vector.tensor_copy(qT[b][:, nt * NT:(nt + 1) * NT], ps[:])

            # ---- attention + staging for AllToAll ----
            a2a_in = dpool.tile([NCORES, CHPC, MBLK], bf16, tag="a2a_in")
            a2a_out = dpool.tile([NCORES, CHPC, MBLK], bf16, tag="a2a_out")

            scale = float(HD) ** -0.5
            for b in range(B):
                for mt in range(M // MBLK):
                    xst = stpool.tile([128, MBLK], bf16, tag="stage")
                    eS = [
                        epool.tile([128, KC, MBLK], bf16, tag="eS", name=f"eS{b}_{mt}")
                        for _ in range(HPC)
                    ]
                    for kc in range(KC):
                        pss = []
                        for h in range(HPC):
                            ps = psS.tile([128, MBLK], fp32, tag="s")
                            # S^T[k, m] for head h; K=64 contraction; the two
                            # heads sit at partitions 0-63 / 64-127 so the PE
                            # row-groups run them concurrently.
                            nc.tensor.matmul(
                                ps[:],
                                lhsT=kT[b][h * HD:(h + 1) * HD, kc * 128:(kc + 1) * 128],
                                rhs=qT[b][h * HD:(h + 1) * HD, mt * MBLK:(mt + 1) * MBLK],
                                start=True,
                                stop=True,
                            )
                            pss.append(ps)
                        for h in range(HPC):
                            nc.scalar.activation(
                                eS[h][:, kc, :], pss[h][:], AF.Exp, scale=scale
                            )
                    for h in range(HPC):
                        for mc in range(MBLK // 128):
                            po = psO.tile([128, NT], fp32, tag="o")
                            for kc in range(KC):
                                nc.tensor.matmul(
                                    po[:, 0:HD + 1],
                                    lhsT=eS[h][:, kc, mc * 128:(mc + 1) * 128],
                                    rhs=vA[b][:, kc, h, :],
                                    start=(kc == 0),
                                    stop=(kc == KC - 1),
                                )
                            rec = spool.tile([128, 1], fp32, tag="rec")
                            nc.vector.reciprocal(rec[:], po[:, HD:HD + 1])
                            osb = spool.tile([128, HD], bf16, tag="osb")
                            nc.vector.tensor_scalar_mul(osb[:], po[:, 0:HD], rec[:])
                            pt = psO.tile([HD, 128], bf16, tag="t", name="pt")
                            nc.tensor.transpose(pt[:], osb[:], ident[:])
                            nc.vector.tensor_copy(
                                xst[h * HD:(h + 1) * HD, mc * 128:(mc + 1) * 128],
                                pt[:],
                            )
                    nc.sync.dma_start(a2a_in[b * 4 + mt], xst[:])

            nc.gpsimd.collective_compute(
                "AllToAll",
                mybir.AluOpType.bypass,
                replica_groups=[list(range(NCORES))],
                ins=[a2a_in[:].opt()],
                outs=[a2a_out[:].opt()],
            )

            # ---- output projection on own row-block ----
            y_sb = ppool.tile([128, NCORES, MBLK], bf16, tag="y")
            for i in range(NCORES):
                nc.sync.dma_start(y_sb[:, i, :], a2a_out[i])
            for oc in range(CC):
                ps = psQ.tile([128, NT], fp32, tag="ps")
                for cc in range(CC):
                    nc.tensor.matmul(
                        ps[:],
                        lhsT=wp_sb[:, cc, oc * 128:(oc + 1) * 128],
                        rhs=y_sb[:, cc, :],
                        start=(cc == 0),
                        stop=(cc == CC - 1),
                    )
                osb = ppool.tile([128, MBLK], fp32, tag="outsb")
                nc.scalar.activation(
                    osb[:], ps[:], AF.Identity, bias=bias_sb[:, oc:oc + 1]
                )
                nc.sync.dma_start(out[oc * 128:(oc + 1) * 128, :], osb[:])

    nc.compile()
    return nc


def _shard_inputs(reference_data, target_data, Wq, Wkv, Wproj, bproj):
    import ml_dtypes

    bf16 = ml_dtypes.bfloat16
    xrefT = np.ascontiguousarray(
        np.asarray(reference_data, dtype=np.float32).transpose(0, 2, 1)
    ).astype(bf16)
    xtgtT = np.ascontiguousarray(
        np.asarray(target_data, dtype=np.float32).transpose(0, 2, 1)
    ).astype(bf16)
    Wq = np.asarray(Wq, dtype=np.float32)
    Wkv = np.asarray(Wkv, dtype=np.float32)
    Wproj_b = np.asarray(Wproj, dtype=np.float32).astype(bf16)
    bproj = np.asarray(bproj, dtype=np.float32)

    in_maps = []
    for c in range(NCORES):
        lo, hi = c * CHPC, (c + 1) * CHPC
        in_maps.append(
            {
                "xrefT": xrefT,
                "xtgtT": xtgtT,
                "wq": Wq[:, lo:hi].astype(bf16),
                "wk": Wkv[:, lo:hi].astype(bf16),
                "wv": Wkv[:, C + lo:C + hi].astype(bf16),
                "wproj": Wproj_b,
                "bproj": bproj,
            }
        )
    return in_maps


def _ensure_ntff_hook():
    """Register the axon NTFF profile hook if the image's antenv lacks it."""
    try:
        import antenv.axon_hooks  # noqa: F401

        return
    except ImportError:
        pass
    import sys
    import types

    import antenv

    mod = types.ModuleType("antenv.axon_hooks")
    state = {"hook": None}
    mod.set_axon_ntff_profile_hook = lambda h: state.__setitem__("hook", h)
    mod.get_axon_ntff_profile_hook = lambda: state["hook"]
    sys.modules["antenv.axon_hooks"] = mod
    antenv.axon_hooks = mod
    try:
        from trn_agent_boot.trn_boot import _ntff_profile_via_ctypes

        mod.set_axon_ntff_profile_hook(
            _ntff_profile_via_ctypes("/opt/axon/libaxon_pjrt.so")
        )
    except Exception:
        pass


def run(inputs: dict, trace: bool = False):
    """Compile (cached), run on 8 cores, return (full_output, BassKernelResults)."""
    from concourse.bass_utils import run_bass_kernel_spmd

    if trace:
        _ensure_ntff_hook()
    nc = _build()
    in_maps = _shard_inputs(**inputs)
    res = run_bass_kernel_spmd(
        nc, in_maps, core_ids=list(range(NCORES)), trace=trace
    )
    full = np.zeros((B, M, C), dtype=np.float32)
    for c in range(NCORES):
        blk = np.asarray(res.results[c]["out"], dtype=np.float32)  # [C, MBLK]
        full[c // 4, (c % 4) * MBLK:(c % 4 + 1) * MBLK, :] = blk.T
    return full, res


def kernel(reference_data, target_data, Wq, Wkv, Wproj, bproj) -> np.ndarray:
    full, _ = run(
        {
            "reference_data": reference_data,
            "target_data": target_data,
            "Wq": Wq,
            "Wkv": Wkv,
            "Wproj": Wproj,
            "bproj": bproj,
        }
    )
    return full
